# revision 21
# baseline (speedup 1.0000x reference)
"""Trainium2 Bass kernel for nn_Encoder_Postnet (length-regulator gather + per-frame linears).

Contract: kernel(**inputs) takes FULL numpy inputs (as produced by
setup_inputs) and returns the FULL [B, T, H] float32 output. Internally the
batch dim is sharded across 8 NeuronCores (pure data parallel, 4 batches per
core); the tiny Linear(1,H) params are replicated.

Per-core dataflow (BPC=4 batches, T=4096 frames, P=512 phonemes, H=512):
  - gather indices (cumsum of align change points) are computed on the host
    (trivial int scan, same spirit as the existing host-side A/W marshaling)
    and shipped as int16 tensors in dma_gather's 16-partition-wrapped layout
  - encoder rows are stored in HBM as fp8(e4m3) and gathered with bulk
    dma_gather ops (1024 rows per op) instead of 128 per-chunk indirect DMAs:
    SWDGE cost is 994ns + 0.34ns/descriptor per *op*, so few big gathers cut
    Q7 descriptor-emission time from ~145us to ~10us, and fp8 halves the
    16MiB/core gather read traffic
  - rank-1 per-frame linears stay as K=11 bf16 matmuls (hi/lo weight split
    keeps pos*w_pos at ~fp32 accuracy), 4 chunks per 4-bank PSUM tile
  - one DVE add per 4-chunk PSUM tile fuses gathered + matmul -> fp16 out
  - fp16 output (halves the 32MiB/core f32 write traffic; ~5e-4 rel err vs
    the 2e-2 gate) written with batched HWDGE DMAs, upcast on host
  - host-permuted "block-of-G" frame order: gather slot i holds frame
    G*(i%128)+i//128, so each SBUF partition owns G consecutive frames and
    output writes are G KiB-contiguous HBM runs (A columns permuted to match)
  - batch-0 chunks 0-7 are expanded on the PE (one-hot fp8 matmuls against
    enc rows 0-511, provably sufficient since idx[t] <= t) + ACT psum copies,
    filling the ~11us gpsimd ucode-library-reload window with real output
  - variable group schedules (small head/tail groups) shorten pipe fill/drain
"""

import sys

if "/opt/trn_rl_repo" not in sys.path:
    sys.path.insert(0, "/opt/trn_rl_repo")

from contextlib import ExitStack

import numpy as np

import concourse.tile as tile
from concourse import bacc, library_config, mybir
from concourse.bass_utils import run_bass_kernel_spmd

B, T, P, H = 32, 4096, 512, 512
NCORES = 8
BPC = B // NCORES            # batches per core
TILE_T = 128                 # frames per chunk (partition dim)
NCHUNK = T // TILE_T         # 32 chunks per batch
GCH = 8                      # chunks per gather/write group
NG = NCHUNK // GCH           # groups per batch
NIDX = GCH * TILE_T          # rows per dma_gather (1024)
QCH = 4                      # chunks per PSUM tile / DVE add
HEAD_NG = 3                  # batch-0 head groups computed on PE (chunks 0-7)
HEAD_CH = 8                  # head chunks
K_MM = 11                    # bf16 matmul contraction (hi/lo split)
F32 = mybir.dt.float32
F16 = mybir.dt.float16
BF16 = mybir.dt.bfloat16
I16 = mybir.dt.int16
FP8 = mybir.dt.float8e4
ADD = mybir.AluOpType.add


# per-batch group-size schedules (chunks per gather/write group): small head
# groups so the first adds start early, small tail groups to shorten the
# drain at the end; sum of each schedule is NCHUNK
def _sched(b):
    if b == 0:
        # first 3 groups are PE-expanded (head): big group first so the
        # last head PSUM tiles have quick ACT copies and free early (only 2
        # PSUM tile bufs); then small gathered groups so the first DVE adds
        # start as early as possible
        return [4, 2, 2, 2, 2, 4, 8, 8]
    if b == BPC - 1:
        return [8, 8, 8, 8, 4, 2, 2][1:]
    return [GCH] * NG


def _emit(ctx: ExitStack, tc: tile.TileContext, enc, idxs, amat, wmat, ohmat,
          out):
    nc = tc.nc
    const = ctx.enter_context(tc.tile_pool(name="const", bufs=1))
    gpool = ctx.enter_context(tc.tile_pool(name="gpool", bufs=6))
    opool = ctx.enter_context(tc.tile_pool(name="opool", bufs=8))
    ppool = ctx.enter_context(tc.tile_pool(name="ppool", bufs=2, space="PSUM"))

    # load the dma_gather ucode library as the first gpsimd instruction so
    # the ~11us reload runs under the input loads, ahead of the first gather
    nc.gpsimd.load_library(library_config.mlp)

    # input loads: the head-gating tensors (OH/ENH on sync, W/A on scalar)
    # go first so the PE head starts ~11us in; ix tiles follow on sync (the
    # first dma_gather can't run before the ucode library reload anyway)
    OH = const.tile([TILE_T, 4 * HEAD_CH * TILE_T], FP8)
    nc.sync.dma_start(OH[:], ohmat[:])
    ENH = const.tile([TILE_T, 4 * H], FP8)
    nc.sync.dma_start(ENH[:].rearrange("k (q h) -> k q h", h=H),
                      enc[0:4 * TILE_T, :].rearrange("(q k) h -> k q h",
                                                     k=TILE_T))
    A_all = const.tile([K_MM, BPC * T], BF16)
    W = const.tile([K_MM, H], BF16)
    nc.scalar.dma_start(W[:], wmat[:])
    nc.scalar.dma_start(A_all[:], amat[:])
    ix_all = const.tile([TILE_T, BPC * (T // 16)], I16)
    for b in range(BPC):
        nc.sync.dma_start(ix_all[:, b * (T // 16):(b + 1) * (T // 16)],
                          idxs[:, b * (T // 16):(b + 1) * (T // 16)])
    As = [A_all[:, b * T:(b + 1) * T] for b in range(BPC)]
    IXs = [ix_all[:, b * (T // 16):(b + 1) * (T // 16)] for b in range(BPC)]

    # --- head: batch-0 chunks 0-7 via PE one-hot expansion (no dma_gather,
    # so they run during the ~11us gpsimd ucode-library reload). Correct for
    # any input: idx[t] <= t and idx <= P-1, so frames 0-1023 only ever read
    # enc rows 0-511. Unused one-hot blocks are all-zero; passes are pruned
    # per group (frames < 256 -> rows < 256 -> 2 passes).
    sched0 = _sched(0)
    s0 = 0
    for g in range(HEAD_NG):
        Gc = sched0[g]
        n = Gc * TILE_T
        # frames of this group are < (s0+Gc)*128, and idx[t] <= t, so only
        # the first min(4, s0+Gc) 128-row one-hot blocks can be nonzero
        npass = min(4, s0 + Gc)
        ov = out[s0 * TILE_T:(s0 + Gc) * TILE_T, :] \
            .rearrange("(p j) h -> p j h", j=Gc)
        q0 = 0
        while q0 < Gc:
            qc = min(QCH, Gc - q0)
            ps = ppool.tile([TILE_T, QCH * H], F32)
            for k in range(qc):
                c = s0 + q0 + k
                for kq in range(min(npass, 4)):
                    nc.tensor.matmul(
                        ps[:, k * H:(k + 1) * H],
                        lhsT=OH[:, (c * 4 + kq) * TILE_T:
                                (c * 4 + kq + 1) * TILE_T],
                        rhs=ENH[:, kq * H:(kq + 1) * H],
                        start=(kq == 0), stop=False)
                nc.tensor.matmul(ps[:, k * H:(k + 1) * H],
                                 lhsT=As[0][:, c * TILE_T:(c + 1) * TILE_T],
                                 rhs=W[:], start=False, stop=True)
            ot = opool.tile([TILE_T, QCH * H], F16)
            nc.scalar.copy(ot[:, :qc * H], ps[:, :qc * H])
            weng = nc.sync if q0 % 2 == 0 else nc.scalar
            weng.dma_start(ov[:, q0:q0 + qc, :],
                           ot[:, :qc * H].rearrange("p (j h) -> p j h", h=H))
            q0 += qc
        s0 += Gc

    i = 0
    for b in range(BPC):
        s0 = 0  # chunk offset within the batch
        for gi, Gc in enumerate(_sched(b)):
            if b == 0 and gi < HEAD_NG:
                s0 += Gc
                continue
            n = Gc * TILE_T
            gt = gpool.tile([TILE_T, GCH * H], FP8)
            nc.gpsimd.dma_gather(
                out_ap=gt[:, :Gc * H].rearrange("p (j h) -> p j h", h=H),
                in_ap=enc[:],
                idxs_ap=IXs[b][:, s0 * 8: s0 * 8 + n // 16],
                num_idxs=n,
                num_idxs_reg=n,
                elem_size=H,
                queue_num=i % 4,
            )
            # block-of-Gc layout (host-permuted idx/A): partition p holds
            # frames Gc*p+j, so each partition writes one contiguous HBM run
            ov = out[b * T + s0 * TILE_T: b * T + (s0 + Gc) * TILE_T, :] \
                .rearrange("(p j) h -> p j h", j=Gc)
            q0 = 0
            while q0 < Gc:
                qc = min(QCH, Gc - q0)
                ps = ppool.tile([TILE_T, QCH * H], F32)
                for k in range(qc):
                    c = s0 + q0 + k
                    nc.tensor.matmul(ps[:, k * H:(k + 1) * H],
                                     lhsT=As[b][:, c * TILE_T:(c + 1) * TILE_T],
                                     rhs=W[:], start=True, stop=True)
                ot = opool.tile([TILE_T, QCH * H], F16)
                nc.vector.tensor_tensor(ot[:, :qc * H],
                                        gt[:, q0 * H:(q0 + qc) * H],
                                        ps[:, :qc * H], op=ADD)
                weng = nc.sync if (i + q0) % 2 == 0 else nc.scalar
                weng.dma_start(ov[:, q0:q0 + qc, :],
                               ot[:, :qc * H].rearrange("p (j h) -> p j h",
                                                        h=H))
                q0 += qc
            s0 += Gc
            i += 1


_CACHED = None


def _build():
    global _CACHED
    if _CACHED is not None:
        return _CACHED
    nc = bacc.Bacc("TRN2", target_bir_lowering=False, debug=False,
                   num_swdge_queues=4)
    enc = nc.dram_tensor("enc", (BPC * P, H), FP8, kind="ExternalInput").ap()
    idxs = nc.dram_tensor("idxs", (TILE_T, BPC * (T // 16)), I16,
                          kind="ExternalInput").ap()
    amat = nc.dram_tensor("amat", (K_MM, BPC * T), BF16,
                          kind="ExternalInput").ap()
    wmat = nc.dram_tensor("wmat", (K_MM, H), BF16, kind="ExternalInput").ap()
    ohmat = nc.dram_tensor("ohmat", (TILE_T, 4 * HEAD_CH * TILE_T), FP8,
                           kind="ExternalInput").ap()
    out = nc.dram_tensor("out", (BPC * T, H), F16, kind="ExternalOutput").ap()

    with tile.TileContext(nc) as tc:
        with ExitStack() as ctx:
            _emit(ctx, tc, enc, idxs, amat, wmat, ohmat, out)
    nc.compile()
    _CACHED = nc
    return nc


def make_in_maps(encoder_out, pitch, beats, align_phone,
                 w_pitch, b_pitch, w_beats, b_beats, w_pos, b_pos):
    import ml_dtypes
    bf16 = ml_dtypes.bfloat16
    fp8 = ml_dtypes.float8_e4m3
    t = np.arange(T, dtype=np.float32)
    t_hi = np.float32(16.0) * np.floor(t / 16.0).astype(np.float32)
    t_lo = t - t_hi
    ones = np.ones(T, np.float32)

    def hilo(w):
        w = np.asarray(w, np.float32)
        hi = w.astype(bf16)
        lo = (w - hi.astype(np.float32)).astype(bf16)
        return hi, lo

    wpos_hi, wpos_lo = hilo(w_pos)
    wpit_hi, wpit_lo = hilo(w_pitch)
    wbea_hi, wbea_lo = hilo(w_beats)
    wmat = np.stack([wpos_hi, wpos_lo, wpos_hi, wpos_lo, wpit_hi, wpit_lo,
                     wbea_hi, wbea_lo,
                     np.asarray(b_pitch, np.float32).astype(bf16),
                     np.asarray(b_beats, np.float32).astype(bf16),
                     np.asarray(b_pos, np.float32).astype(bf16)])

    # host-side gather indices: idx = cumsum of change points, offset by the
    # batch's row base in the flattened [BPC*P, H] enc tensor, then wrapped
    # into dma_gather's (16-partition, replicated) int16 layout
    align = np.asarray(align_phone, np.int32)
    change = np.concatenate(
        [np.zeros((B, 1), np.int32),
         (align[:, 1:] != align[:, :-1]).astype(np.int32)], axis=1)
    idx = np.minimum(np.cumsum(change, axis=1), P - 1)  # [B, T]

    pitch = np.asarray(pitch, np.float32)
    beats = np.asarray(beats, np.float32)

    in_maps = []
    for r in range(NCORES):
        s = slice(r * BPC, (r + 1) * BPC)
        amat = np.empty((K_MM, BPC * T), np.float32)
        for b in range(BPC):
            gb = r * BPC + b
            a = np.stack([t_hi, t_hi, t_lo, t_lo, pitch[gb], pitch[gb],
                          beats[gb], beats[gb], ones, ones, ones])
            # permute columns to match the block-of-Gc frame layout: matmul
            # lhsT column p of chunk-slot j must be frame Gc*p+j of its group
            f0 = 0
            for gc in _sched(b):
                n = gc * TILE_T
                a[:, f0:f0 + n] = (a[:, f0:f0 + n]
                                   .reshape(K_MM, TILE_T, gc)
                                   .transpose(0, 2, 1).reshape(K_MM, n))
                f0 += n
            amat[:, b * T:(b + 1) * T] = a
        # block-of-Gc permutation: gather slot i of a group fetches frame
        # Gc*(i%128) + i//128, so partition p receives Gc consecutive frames
        idxw = np.empty((TILE_T, BPC * (T // 16)), np.int16)
        for b in range(BPC):
            fperm = np.empty(T, np.int64)
            f0 = 0
            for gc in _sched(b):
                n = gc * TILE_T
                i_ = np.arange(n)
                fperm[f0:f0 + n] = f0 + gc * (i_ % TILE_T) + i_ // TILE_T
                f0 += n
            gidx = (idx[r * BPC + b] + b * P).astype(np.int16)[fperm]
            wrapped = gidx.reshape(T // 16, 16).T               # [16, T/16]
            idxw[:, b * (T // 16):(b + 1) * (T // 16)] = np.tile(wrapped,
                                                                (8, 1))
        # one-hot matrices for the PE-expanded head chunks of local batch 0:
        # ohmat[kp, (c*4+kq)*128+p] = (idx0[frame(c,p)] == kq*128+kp)
        # lhsT column (c*128+p) holds frame f0 + gc*p + j with c = f0/128+j
        idx0 = idx[r * BPC]
        colframe = np.empty(HEAD_CH * TILE_T, np.int64)
        f0 = 0
        for gc in _sched(0)[:HEAD_NG]:
            n_ = gc * TILE_T
            jj = np.arange(n_) // TILE_T
            pp = np.arange(n_) % TILE_T
            colframe[f0:f0 + n_] = f0 + gc * pp + jj
            f0 += n_
        kk = idx0[colframe]                                   # [1024]
        oh_full = (kk[None, :] == np.arange(P)[:, None])      # [512, 1024]
        ohmat_ = (oh_full.reshape(4, TILE_T, HEAD_CH, TILE_T)
                  .transpose(1, 2, 0, 3)
                  .reshape(TILE_T, 4 * HEAD_CH * TILE_T)).astype(fp8)
        in_maps.append({
            "enc": np.ascontiguousarray(
                encoder_out[s], np.float32).reshape(BPC * P, H).astype(fp8),
            "idxs": idxw,
            "amat": amat.astype(bf16),
            "wmat": wmat,
            "ohmat": ohmat_,
        })
    return in_maps


def _run_in_subprocess(kwargs):
    """Fallback for a wedged in-process PJRT client: re-run this module in a
    fresh interpreter (fresh device boot), passing inputs via pickle."""
    import os
    import pickle
    import subprocess
    import tempfile

    with tempfile.TemporaryDirectory() as td:
        inp = os.path.join(td, "in.pkl")
        outp = os.path.join(td, "out.npy")
        with open(inp, "wb") as f:
            pickle.dump(kwargs, f)
        code = (
            "import pickle, numpy as np, importlib.util\n"
            f"spec = importlib.util.spec_from_file_location('k', {__file__!r})\n"
            "m = importlib.util.module_from_spec(spec)\n"
            "spec.loader.exec_module(m)\n"
            f"ins = pickle.load(open({inp!r}, 'rb'))\n"
            f"np.save({outp!r}, m.kernel(**ins, _no_fallback=True))\n"
        )
        subprocess.run([sys.executable, "-c", code], check=True, timeout=1700)
        return np.load(outp)


def kernel(encoder_out, pitch, beats, w_pitch, b_pitch, w_beats, b_beats,
           w_pos, b_pos, align_phone, _trace=False, _no_fallback=False):
    kwargs = dict(encoder_out=np.asarray(encoder_out),
                  pitch=np.asarray(pitch), beats=np.asarray(beats),
                  w_pitch=np.asarray(w_pitch), b_pitch=np.asarray(b_pitch),
                  w_beats=np.asarray(w_beats), b_beats=np.asarray(b_beats),
                  w_pos=np.asarray(w_pos), b_pos=np.asarray(b_pos),
                  align_phone=np.asarray(align_phone))
    nc = _build()
    in_maps = make_in_maps(encoder_out, pitch, beats, align_phone,
                           w_pitch, b_pitch, w_beats, b_beats, w_pos, b_pos)

    def attempt():
        # materialize eagerly so device failures surface inside the guard
        res = run_bass_kernel_spmd(nc, in_maps, core_ids=list(range(NCORES)),
                                   trace=_trace)
        return res, np.concatenate(
            [np.asarray(res.results[r]["out"]).astype(np.float32)
             .reshape(BPC, T, H) for r in range(NCORES)], axis=0)

    import time
    res = out = None
    for i in range(2):
        try:
            res, out = attempt()
            break
        except Exception:
            # rare flaky device hang (NRT_EXEC_UNIT_UNRECOVERABLE)
            time.sleep(5.0)
    if out is None:
        if _no_fallback:
            res, out = attempt()
        else:
            # fresh interpreter = fresh PJRT client + device reset
            try:
                return _run_in_subprocess(kwargs)
            except Exception:
                time.sleep(10.0)
                return _run_in_subprocess(kwargs)
    if _trace:
        kernel.last_results = res
    return out


# revision 22
# speedup vs baseline: 1.0372x; 1.0372x over previous
"""Trainium2 Bass kernel for nn_Encoder_Postnet (length-regulator gather + per-frame linears).

Contract: kernel(**inputs) takes FULL numpy inputs (as produced by
setup_inputs) and returns the FULL [B, T, H] float32 output. Internally the
batch dim is sharded across 8 NeuronCores (pure data parallel, 4 batches per
core); the tiny Linear(1,H) params are replicated.

Per-core dataflow (BPC=4 batches, T=4096 frames, P=512 phonemes, H=512):
  - gather indices (cumsum of align change points) are computed on the host
    (trivial int scan, same spirit as the existing host-side A/W marshaling)
    and shipped as int16 tensors in dma_gather's 16-partition-wrapped layout
  - encoder rows are stored in HBM as fp8(e4m3) and gathered with bulk
    dma_gather ops (1024 rows per op) instead of 128 per-chunk indirect DMAs:
    SWDGE cost is 994ns + 0.34ns/descriptor per *op*, so few big gathers cut
    Q7 descriptor-emission time from ~145us to ~10us, and fp8 halves the
    16MiB/core gather read traffic
  - rank-1 per-frame linears stay as K=11 bf16 matmuls (hi/lo weight split
    keeps pos*w_pos at ~fp32 accuracy), 4 chunks per 4-bank PSUM tile
  - one DVE add per 4-chunk PSUM tile fuses gathered + matmul -> fp16 out
  - fp16 output (halves the 32MiB/core f32 write traffic; ~5e-4 rel err vs
    the 2e-2 gate) written with batched HWDGE DMAs, upcast on host
  - host-permuted "block-of-G" frame order: gather slot i holds frame
    G*(i%128)+i//128, so each SBUF partition owns G consecutive frames and
    output writes are G KiB-contiguous HBM runs (A columns permuted to match)
  - batch-0 chunks 0-7 are expanded on the PE (one-hot fp8 matmuls against
    enc rows 0-511, provably sufficient since idx[t] <= t) + ACT psum copies,
    filling the ~11us gpsimd ucode-library-reload window with real output
  - variable group schedules (small head/tail groups) shorten pipe fill/drain
"""

import sys

if "/opt/trn_rl_repo" not in sys.path:
    sys.path.insert(0, "/opt/trn_rl_repo")

from contextlib import ExitStack

import numpy as np

import concourse.tile as tile
from concourse import bacc, library_config, mybir
from concourse.bass_utils import run_bass_kernel_spmd

B, T, P, H = 32, 4096, 512, 512
NCORES = 8
BPC = B // NCORES            # batches per core
TILE_T = 128                 # frames per chunk (partition dim)
NCHUNK = T // TILE_T         # 32 chunks per batch
GCH = 8                      # chunks per gather/write group
NG = NCHUNK // GCH           # groups per batch
NIDX = GCH * TILE_T          # rows per dma_gather (1024)
QCH = 4                      # chunks per PSUM tile / DVE add
HEAD_NG = 3                  # batch-0 head groups computed on PE (chunks 0-7)
HEAD_CH = 8                  # head chunks
K_MM = 11                    # bf16 matmul contraction (hi/lo split)
F32 = mybir.dt.float32
F16 = mybir.dt.float16
BF16 = mybir.dt.bfloat16
I16 = mybir.dt.int16
FP8 = mybir.dt.float8e4
ADD = mybir.AluOpType.add


# per-batch group-size schedules (chunks per gather/write group): small head
# groups so the first adds start early, small tail groups to shorten the
# drain at the end; sum of each schedule is NCHUNK
def _sched(b):
    if b == 0:
        # first 3 groups are PE-expanded (head); the first *gathered* groups
        # are small again so the first DVE adds start as early as possible
        return [2, 2, 4, 2, 2, 4, 8, 8]
    if b == BPC - 1:
        return [8, 8, 8, 8, 4, 2, 2][1:]
    return [GCH] * NG


def _emit(ctx: ExitStack, tc: tile.TileContext, enc, idxs, amat, wmat, ohmat,
          out):
    nc = tc.nc
    const = ctx.enter_context(tc.tile_pool(name="const", bufs=1))
    gpool = ctx.enter_context(tc.tile_pool(name="gpool", bufs=6))
    opool = ctx.enter_context(tc.tile_pool(name="opool", bufs=8))
    ppool = ctx.enter_context(tc.tile_pool(name="ppool", bufs=2, space="PSUM"))

    # load the dma_gather ucode library as the first gpsimd instruction so
    # the ~11us reload runs under the input loads, ahead of the first gather
    nc.gpsimd.load_library(library_config.mlp)

    # input loads: the head-gating tensors (OH/ENH on sync, W/A on scalar)
    # go first so the PE head starts ~11us in; ix tiles follow on sync (the
    # first dma_gather can't run before the ucode library reload anyway)
    OH = const.tile([TILE_T, 4 * HEAD_CH * TILE_T], FP8)
    nc.sync.dma_start(OH[:], ohmat[:])
    ENH = const.tile([TILE_T, 4 * H], FP8)
    nc.sync.dma_start(ENH[:].rearrange("k (q h) -> k q h", h=H),
                      enc[0:4 * TILE_T, :].rearrange("(q k) h -> k q h",
                                                     k=TILE_T))
    A_all = const.tile([K_MM, BPC * T], BF16)
    W = const.tile([K_MM, H], BF16)
    nc.scalar.dma_start(W[:], wmat[:])
    nc.scalar.dma_start(A_all[:], amat[:])
    ix_all = const.tile([TILE_T, BPC * (T // 16)], I16)
    for b in range(BPC):
        nc.sync.dma_start(ix_all[:, b * (T // 16):(b + 1) * (T // 16)],
                          idxs[:, b * (T // 16):(b + 1) * (T // 16)])
    As = [A_all[:, b * T:(b + 1) * T] for b in range(BPC)]
    IXs = [ix_all[:, b * (T // 16):(b + 1) * (T // 16)] for b in range(BPC)]

    # --- head: batch-0 chunks 0-7 via PE one-hot expansion (no dma_gather,
    # so they run during the ~11us gpsimd ucode-library reload). Correct for
    # any input: idx[t] <= t and idx <= P-1, so frames 0-1023 only ever read
    # enc rows 0-511. Unused one-hot blocks are all-zero; passes are pruned
    # per group (frames < 256 -> rows < 256 -> 2 passes).
    sched0 = _sched(0)
    s0 = 0
    for g in range(HEAD_NG):
        Gc = sched0[g]
        n = Gc * TILE_T
        # frames of this group are < (s0+Gc)*128, and idx[t] <= t, so only
        # the first min(4, s0+Gc) 128-row one-hot blocks can be nonzero
        npass = min(4, s0 + Gc)
        ov = out[s0 * TILE_T:(s0 + Gc) * TILE_T, :] \
            .rearrange("(p j) h -> p j h", j=Gc)
        q0 = 0
        while q0 < Gc:
            qc = min(QCH, Gc - q0)
            ps = ppool.tile([TILE_T, QCH * H], F32)
            for k in range(qc):
                c = s0 + q0 + k
                for kq in range(min(npass, 4)):
                    nc.tensor.matmul(
                        ps[:, k * H:(k + 1) * H],
                        lhsT=OH[:, (c * 4 + kq) * TILE_T:
                                (c * 4 + kq + 1) * TILE_T],
                        rhs=ENH[:, kq * H:(kq + 1) * H],
                        start=(kq == 0), stop=False)
                nc.tensor.matmul(ps[:, k * H:(k + 1) * H],
                                 lhsT=As[0][:, c * TILE_T:(c + 1) * TILE_T],
                                 rhs=W[:], start=False, stop=True)
            ot = opool.tile([TILE_T, QCH * H], F16)
            nc.scalar.copy(ot[:, :qc * H], ps[:, :qc * H])
            weng = nc.sync if q0 % 2 == 0 else nc.scalar
            weng.dma_start(ov[:, q0:q0 + qc, :],
                           ot[:, :qc * H].rearrange("p (j h) -> p j h", h=H))
            q0 += qc
        s0 += Gc

    i = 0
    for b in range(BPC):
        s0 = 0  # chunk offset within the batch
        for gi, Gc in enumerate(_sched(b)):
            if b == 0 and gi < HEAD_NG:
                s0 += Gc
                continue
            n = Gc * TILE_T
            gt = gpool.tile([TILE_T, GCH * H], FP8)
            nc.gpsimd.dma_gather(
                out_ap=gt[:, :Gc * H].rearrange("p (j h) -> p j h", h=H),
                in_ap=enc[:],
                idxs_ap=IXs[b][:, s0 * 8: s0 * 8 + n // 16],
                num_idxs=n,
                num_idxs_reg=n,
                elem_size=H,
                queue_num=i % 4,
            )
            # block-of-Gc layout (host-permuted idx/A): partition p holds
            # frames Gc*p+j, so each partition writes one contiguous HBM run
            ov = out[b * T + s0 * TILE_T: b * T + (s0 + Gc) * TILE_T, :] \
                .rearrange("(p j) h -> p j h", j=Gc)
            q0 = 0
            while q0 < Gc:
                qc = min(QCH, Gc - q0)
                ps = ppool.tile([TILE_T, QCH * H], F32)
                for k in range(qc):
                    c = s0 + q0 + k
                    nc.tensor.matmul(ps[:, k * H:(k + 1) * H],
                                     lhsT=As[b][:, c * TILE_T:(c + 1) * TILE_T],
                                     rhs=W[:], start=True, stop=True)
                ot = opool.tile([TILE_T, QCH * H], F16)
                nc.vector.tensor_tensor(ot[:, :qc * H],
                                        gt[:, q0 * H:(q0 + qc) * H],
                                        ps[:, :qc * H], op=ADD)
                weng = nc.sync if (i + q0) % 2 == 0 else nc.scalar
                weng.dma_start(ov[:, q0:q0 + qc, :],
                               ot[:, :qc * H].rearrange("p (j h) -> p j h",
                                                        h=H))
                q0 += qc
            s0 += Gc
            i += 1


_CACHED = None


def _build():
    global _CACHED
    if _CACHED is not None:
        return _CACHED
    nc = bacc.Bacc("TRN2", target_bir_lowering=False, debug=False,
                   num_swdge_queues=4)
    enc = nc.dram_tensor("enc", (BPC * P, H), FP8, kind="ExternalInput").ap()
    idxs = nc.dram_tensor("idxs", (TILE_T, BPC * (T // 16)), I16,
                          kind="ExternalInput").ap()
    amat = nc.dram_tensor("amat", (K_MM, BPC * T), BF16,
                          kind="ExternalInput").ap()
    wmat = nc.dram_tensor("wmat", (K_MM, H), BF16, kind="ExternalInput").ap()
    ohmat = nc.dram_tensor("ohmat", (TILE_T, 4 * HEAD_CH * TILE_T), FP8,
                           kind="ExternalInput").ap()
    out = nc.dram_tensor("out", (BPC * T, H), F16, kind="ExternalOutput").ap()

    with tile.TileContext(nc) as tc:
        with ExitStack() as ctx:
            _emit(ctx, tc, enc, idxs, amat, wmat, ohmat, out)
    nc.compile()
    _CACHED = nc
    return nc


def make_in_maps(encoder_out, pitch, beats, align_phone,
                 w_pitch, b_pitch, w_beats, b_beats, w_pos, b_pos):
    import ml_dtypes
    bf16 = ml_dtypes.bfloat16
    fp8 = ml_dtypes.float8_e4m3
    t = np.arange(T, dtype=np.float32)
    t_hi = np.float32(16.0) * np.floor(t / 16.0).astype(np.float32)
    t_lo = t - t_hi
    ones = np.ones(T, np.float32)

    def hilo(w):
        w = np.asarray(w, np.float32)
        hi = w.astype(bf16)
        lo = (w - hi.astype(np.float32)).astype(bf16)
        return hi, lo

    wpos_hi, wpos_lo = hilo(w_pos)
    wpit_hi, wpit_lo = hilo(w_pitch)
    wbea_hi, wbea_lo = hilo(w_beats)
    wmat = np.stack([wpos_hi, wpos_lo, wpos_hi, wpos_lo, wpit_hi, wpit_lo,
                     wbea_hi, wbea_lo,
                     np.asarray(b_pitch, np.float32).astype(bf16),
                     np.asarray(b_beats, np.float32).astype(bf16),
                     np.asarray(b_pos, np.float32).astype(bf16)])

    # host-side gather indices: idx = cumsum of change points, offset by the
    # batch's row base in the flattened [BPC*P, H] enc tensor, then wrapped
    # into dma_gather's (16-partition, replicated) int16 layout
    align = np.asarray(align_phone, np.int32)
    change = np.concatenate(
        [np.zeros((B, 1), np.int32),
         (align[:, 1:] != align[:, :-1]).astype(np.int32)], axis=1)
    idx = np.minimum(np.cumsum(change, axis=1), P - 1)  # [B, T]

    pitch = np.asarray(pitch, np.float32)
    beats = np.asarray(beats, np.float32)

    in_maps = []
    for r in range(NCORES):
        s = slice(r * BPC, (r + 1) * BPC)
        amat = np.empty((K_MM, BPC * T), np.float32)
        for b in range(BPC):
            gb = r * BPC + b
            a = np.stack([t_hi, t_hi, t_lo, t_lo, pitch[gb], pitch[gb],
                          beats[gb], beats[gb], ones, ones, ones])
            # permute columns to match the block-of-Gc frame layout: matmul
            # lhsT column p of chunk-slot j must be frame Gc*p+j of its group
            f0 = 0
            for gc in _sched(b):
                n = gc * TILE_T
                a[:, f0:f0 + n] = (a[:, f0:f0 + n]
                                   .reshape(K_MM, TILE_T, gc)
                                   .transpose(0, 2, 1).reshape(K_MM, n))
                f0 += n
            amat[:, b * T:(b + 1) * T] = a
        # block-of-Gc permutation: gather slot i of a group fetches frame
        # Gc*(i%128) + i//128, so partition p receives Gc consecutive frames
        idxw = np.empty((TILE_T, BPC * (T // 16)), np.int16)
        for b in range(BPC):
            fperm = np.empty(T, np.int64)
            f0 = 0
            for gc in _sched(b):
                n = gc * TILE_T
                i_ = np.arange(n)
                fperm[f0:f0 + n] = f0 + gc * (i_ % TILE_T) + i_ // TILE_T
                f0 += n
            gidx = (idx[r * BPC + b] + b * P).astype(np.int16)[fperm]
            wrapped = gidx.reshape(T // 16, 16).T               # [16, T/16]
            idxw[:, b * (T // 16):(b + 1) * (T // 16)] = np.tile(wrapped,
                                                                (8, 1))
        # one-hot matrices for the PE-expanded head chunks of local batch 0:
        # ohmat[kp, (c*4+kq)*128+p] = (idx0[frame(c,p)] == kq*128+kp)
        # lhsT column (c*128+p) holds frame f0 + gc*p + j with c = f0/128+j
        idx0 = idx[r * BPC]
        colframe = np.empty(HEAD_CH * TILE_T, np.int64)
        f0 = 0
        for gc in _sched(0)[:HEAD_NG]:
            n_ = gc * TILE_T
            jj = np.arange(n_) // TILE_T
            pp = np.arange(n_) % TILE_T
            colframe[f0:f0 + n_] = f0 + gc * pp + jj
            f0 += n_
        kk = idx0[colframe]                                   # [1024]
        oh_full = (kk[None, :] == np.arange(P)[:, None])      # [512, 1024]
        ohmat_ = (oh_full.reshape(4, TILE_T, HEAD_CH, TILE_T)
                  .transpose(1, 2, 0, 3)
                  .reshape(TILE_T, 4 * HEAD_CH * TILE_T)).astype(fp8)
        in_maps.append({
            "enc": np.ascontiguousarray(
                encoder_out[s], np.float32).reshape(BPC * P, H).astype(fp8),
            "idxs": idxw,
            "amat": amat.astype(bf16),
            "wmat": wmat,
            "ohmat": ohmat_,
        })
    return in_maps


def _run_in_subprocess(kwargs):
    """Fallback for a wedged in-process PJRT client: re-run this module in a
    fresh interpreter (fresh device boot), passing inputs via pickle."""
    import os
    import pickle
    import subprocess
    import tempfile

    with tempfile.TemporaryDirectory() as td:
        inp = os.path.join(td, "in.pkl")
        outp = os.path.join(td, "out.npy")
        with open(inp, "wb") as f:
            pickle.dump(kwargs, f)
        code = (
            "import pickle, numpy as np, importlib.util\n"
            f"spec = importlib.util.spec_from_file_location('k', {__file__!r})\n"
            "m = importlib.util.module_from_spec(spec)\n"
            "spec.loader.exec_module(m)\n"
            f"ins = pickle.load(open({inp!r}, 'rb'))\n"
            f"np.save({outp!r}, m.kernel(**ins, _no_fallback=True))\n"
        )
        subprocess.run([sys.executable, "-c", code], check=True, timeout=1700)
        return np.load(outp)


def kernel(encoder_out, pitch, beats, w_pitch, b_pitch, w_beats, b_beats,
           w_pos, b_pos, align_phone, _trace=False, _no_fallback=False):
    kwargs = dict(encoder_out=np.asarray(encoder_out),
                  pitch=np.asarray(pitch), beats=np.asarray(beats),
                  w_pitch=np.asarray(w_pitch), b_pitch=np.asarray(b_pitch),
                  w_beats=np.asarray(w_beats), b_beats=np.asarray(b_beats),
                  w_pos=np.asarray(w_pos), b_pos=np.asarray(b_pos),
                  align_phone=np.asarray(align_phone))
    nc = _build()
    in_maps = make_in_maps(encoder_out, pitch, beats, align_phone,
                           w_pitch, b_pitch, w_beats, b_beats, w_pos, b_pos)

    def attempt():
        # materialize eagerly so device failures surface inside the guard
        res = run_bass_kernel_spmd(nc, in_maps, core_ids=list(range(NCORES)),
                                   trace=_trace)
        return res, np.concatenate(
            [np.asarray(res.results[r]["out"]).astype(np.float32)
             .reshape(BPC, T, H) for r in range(NCORES)], axis=0)

    import time
    res = out = None
    for i in range(2):
        try:
            res, out = attempt()
            break
        except Exception:
            # rare flaky device hang (NRT_EXEC_UNIT_UNRECOVERABLE)
            time.sleep(5.0)
    if out is None:
        if _no_fallback:
            res, out = attempt()
        else:
            # fresh interpreter = fresh PJRT client + device reset
            try:
                return _run_in_subprocess(kwargs)
            except Exception:
                time.sleep(10.0)
                return _run_in_subprocess(kwargs)
    if _trace:
        kernel.last_results = res
    return out
